# revision 69
# baseline (speedup 1.0000x reference)
"""Trainium2 Bass kernel for nn_DecorrelationPatch2d.

reference = fold(unfold(x) * R.sum(1)) / fold(unfold(ones)) collapses to
out[n,c,h,w] = x[n,c,h,w] * W[c,h,w]: the per-feature scaling is
elementwise in the unfolded domain, so fold/unfold reduce to a per-pixel
window-average of s = R.sum(1).reshape(C,3,3):

  W[c,h,w] = mean over valid offsets (i,j) of s[c,i,j]
           = (Bh' @ S_c @ Bw'^T)[h,w]        (rank-3 separable)

with Bh'[h,i] = [i in Vh(h)]/|Vh(h)|, Bw'[w,j] = [j in Vw(w)]/|Vw(w)|.

Sharding: channels C=64 split 8-per-core across 8 cores; each core does
an elementwise multiply of its [N=8, 8, 128, 128] x-shard. Device layout
is [H=128(partitions), N, C_s, W] so every DMA moves contiguous runs per
partition.

The kernel is DMA-bound, so the data path is bf16 end to end: the host
rounds x to bf16 (per-element rel err <= 2^-9, far inside the 2e-2
tolerance; bf16 keeps f32's exponent range so tiny values lose no
relative precision, unlike fp16), the device streams bf16 in and out
(half the HBM traffic of f32), and the host widens the result back to
f32. The W map is generated on-device: the host ships one [3, 1152]
bf16 table ([G_0 | .. | G_7 | Bh'^T], G_c = S_c @ Bw'^T) and 8 k=3 PE
matmuls produce the 8 channels' W into PSUM; the Activation engine
narrows it to a bf16 SBUF tile (two halves) so the DVE multiplies run
in 2x (16-bit) mode.

Pipeline shape (default variant "v10"; all timings from the TimelineSim
cost model, which is also what the harness reports as HW exec time):
the 8 input chunks lead the shared-HWDGE queue and stream back to back
on the DMA engines from the earliest possible slot; the small wt load
rides the Pool/SWDGE ring so it costs no HWDGE cadence and slots into
the queue backlog. The W chain (matmuls -> act-copy halves) is gated
only on the wt DMA, so the first (256-col) multiply and its Pool-issued
output DMA are ready right as the input stream drains; the next few
output DMAs issue from the then-idle SP queue (shorter DGE delay), the
rest from Act. The DMA engines stay saturated through the in->out
transition and the model lands at 15.6us vs the 23.3us it charges for
the same traffic in f32.

Raw Bass (no Tile): this container's walrus rejects >1 sync-wait per
instruction, which Tile's scheduler/drain freely emit. Manual semaphores
below keep every instruction at <=1 wait; a DVE spacer copy carries the
chunk-1 wait so the first multiply needs only the W-ready wait. A
sem-clear tail makes the loaded NEFF safely re-executable (the PJRT path
keeps it loaded across kernel() calls).

DMA completion correctness: a `.then_inc(sem, 16)` DMA completes as 16
independent per-ring sub-DMAs, so a shared counter only bounds the
TOTAL inc count — a straggler ring can leave one chunk's partition
slice unwritten while later chunks' rings push a shared counter past
the gate (observed on hardware as stale partitions 96..103 = ring 12's
slice). Every data gate therefore uses the producing DMA's OWN
semaphore at full count: each multiply of chunk n gates on chunk n+1's
sem==16 (per-ring in-order processing then gives chunk n a full
transfer of byte-landing margin), a 64-col re-read fence with its own
sem covers chunk 7, and the PE gates on the wt load's sem==16 with a
throwaway 1-column matmul soaking up the remaining tens-of-ns
sem-vs-byte skew before the real matmuls read the table.
"""

import numpy as np
import ml_dtypes

import concourse.bass as bass
from concourse import mybir
from concourse.bass_utils import run_bass_kernel_spmd

N, C, H, W = 8, 64, 128, 128
KH = KW = 3
NCORES = 8
CS = C // NCORES  # channels per core
FW = CS * W  # free-dim elems per (h, n) slice = 1024
FX = N * FW  # free-dim elems per partition of the x shard = 8192
KF = CS * KH  # fused matmul contraction dim = 24
WTC = H + FW  # wt columns: lhsT block (128) + rhs block (1024) = 1152
S0 = 256  # first multiply/output split point (512B/partition in bf16)

BF16 = ml_dtypes.bfloat16

_NC_CACHE = {}


def _build_nc(variant="v10"):
    if variant in _NC_CACHE:
        return _NC_CACHE[variant]
    if variant.startswith(("v5", "v6", "v7", "v8", "v9", "v10", "v11", "v12")):
        nc = _build_nc_v5(variant)
        _NC_CACHE[variant] = nc
        return nc
    f32 = mybir.dt.float32
    bf16 = mybir.dt.bfloat16
    nc = bass.Bass()
    xt = nc.dram_tensor("xt", [H, FX], bf16, kind="ExternalInput")
    wt = nc.dram_tensor("wt", [KF, WTC], bf16, kind="ExternalInput")
    out = nc.dram_tensor("out", [H, FX], bf16, kind="ExternalOutput")

    # v3 family: wt is DMA'd twice back to back on the same ring; the PE
    # gates on the second load's sem, which proves the first load's
    # (identical) bytes are visible — a race-free fast W chain.
    # v2a/v2b: wt loaded once, PE gated on its own sem behind a throwaway
    # matmul (raced on HW — kept only for reference).
    # v2c: conservative chunk-0 gate.
    # v4: second wt load issued from the Pool engine (SWDGE ring — its
    # descriptor generation runs on the otherwise-idle Pool engine, so it
    # does not consume the shared HWDGE's ~650ns/instruction cadence).
    double_wt = variant in ("v3", "v3a", "v4")
    pool_wt2 = variant == "v4"
    fast_w = variant in ("v2a", "v2b")
    pool_first_out = variant in ("v2b", "v3", "v4")

    with (
        nc.Block() as block,
        nc.semaphore("in_sem") as in_sem,
        nc.semaphore("mm_sem") as mm_sem,
        nc.semaphore("cp_sem") as cp_sem,
        nc.semaphore("comp_sem") as comp_sem,
        nc.semaphore("out_sem") as out_sem,
        nc.sbuf_tensor("wib", [KF, WTC], bf16) as wib,
        nc.sbuf_tensor("spacer", [1, 1], bf16) as spacer,
        nc.sbuf_tensor("fence_buf", [1, 1], bf16) as fence_buf,
        nc.sbuf_tensor("xbuf", [H, FX], bf16) as xbuf,
        nc.sbuf_tensor("ybuf", [H, FX], bf16) as ybuf,
        nc.sbuf_tensor("wsb", [H, FW], bf16) as wsb,
        nc.psum_tensor("WP", [H, FW], f32) as WP,
    ):

        n_wt = 2 if double_wt else 1  # leading wt DMAs before the x chunks

        @block.sync
        def _(sync: bass.BassEngine):
            for _ in range(1 if pool_wt2 else n_wt):
                sync.dma_start(out=wib[:, :], in_=wt[:, :]).then_inc(in_sem, 16)
            for n in range(N):
                sl = slice(n * FW, (n + 1) * FW)
                sync.dma_start(out=xbuf[:, sl], in_=xt[:, sl]).then_inc(in_sem, 16)
            # fence: re-read the last chunk's tail element so chunk 7 also
            # gets a ring-ordered completion margin
            sync.dma_start(
                out=fence_buf[:, :], in_=xbuf[H - 1 : H, FX - 1 : FX]
            ).then_inc(in_sem, 16)

        @block.tensor
        def _(tensor: bass.BassEngine):
            half = FW // 2
            if double_wt:
                # second wt load's completion proves the first load's
                # (identical) bytes are visible: race-free by value identity
                tensor.wait_ge(in_sem, 32)
            elif fast_w:
                # gate on the wt DMA itself; the throwaway 1-column matmul
                # below soaks up the sem-vs-byte-visibility skew so the real
                # matmuls read wib well after its bytes landed
                tensor.wait_ge(in_sem, 16)
                tensor.matmul(
                    WP[:, 0:1],
                    wib[:, 0:H],
                    wib[:, H : H + 1],
                    start=True,
                    stop=True,
                )
            else:
                # gate on x chunk 0: ring-ordered after the wt table, so its
                # completion gives wib's bytes a full transfer of margin
                tensor.wait_ge(in_sem, 16 * (n_wt + 1))
            for m in range(2):
                # W[:, m*512:(m+1)*512] = lhsT^T @ rhs_half -> PSUM f32;
                # 512-col matmuls keep each output inside one PSUM bank
                tensor.matmul(
                    WP[:, m * half : (m + 1) * half],
                    wib[:, 0:H],
                    wib[:, H + m * half : H + (m + 1) * half],
                    start=True,
                    stop=True,
                ).then_inc(mm_sem, 1)

        @block.vector
        def _(vector: bass.BassEngine):
            # spacer carries the chunk-1 wait (ring-ordered margin for chunk
            # 0's bytes) so the first multiply only needs the W-ready wait
            vector.wait_ge(in_sem, 16 * (n_wt + 2))
            vector.tensor_copy(spacer[:, :], xbuf[0:1, FW - 1 : FW])
            vector.wait_ge(cp_sem, 1)
            vector.tensor_mul(
                ybuf[:, 0:S0], xbuf[:, 0:S0], wsb[:, 0:S0]
            ).then_inc(comp_sem, 1)
            vector.tensor_mul(
                ybuf[:, S0:FW], xbuf[:, S0:FW], wsb[:, S0:FW]
            ).then_inc(comp_sem, 1)
            for n in range(1, N):
                sl = slice(n * FW, (n + 1) * FW)
                # wait one chunk AHEAD (chunk n+1 / the fence) for margin
                vector.wait_ge(in_sem, 16 * (n_wt + n + 2))
                vector.tensor_mul(ybuf[:, sl], xbuf[:, sl], wsb[:, :]).then_inc(
                    comp_sem, 1
                )

        if pool_wt2 or pool_first_out:

            @block.gpsimd
            def _(gpsimd: bass.BassEngine):
                if pool_wt2:
                    gpsimd.dma_start(out=wib[:, :], in_=wt[:, :]).then_inc(
                        in_sem, 16
                    )
                if pool_first_out:
                    gpsimd.wait_ge(comp_sem, 1)
                    gpsimd.dma_start(
                        out=out[:, 0:S0], in_=ybuf[:, 0:S0]
                    ).then_inc(out_sem, 16)

        @block.scalar
        def _(scalar: bass.BassEngine):
            scalar.wait_ge(mm_sem, 2)
            # narrow W to bf16 in SBUF (Act engine, f32 PSUM read) so the
            # multiplies run in DVE 2x mode and DVE never stalls on this copy
            scalar.activation(
                wsb[:, :], WP[:, :], mybir.ActivationFunctionType.Copy
            ).then_inc(cp_sem, 1)
            if not pool_first_out:
                scalar.wait_ge(comp_sem, 1)
                scalar.dma_start(out=out[:, 0:S0], in_=ybuf[:, 0:S0]).then_inc(
                    out_sem, 16
                )
            scalar.wait_ge(comp_sem, 2)
            scalar.dma_start(out=out[:, S0:FW], in_=ybuf[:, S0:FW]).then_inc(
                out_sem, 16
            )
            for n in range(1, N):
                sl = slice(n * FW, (n + 1) * FW)
                scalar.wait_ge(comp_sem, n + 2)
                scalar.dma_start(out=out[:, sl], in_=ybuf[:, sl]).then_inc(
                    out_sem, 16
                )
            scalar.wait_ge(out_sem, 16 * (N + 1))
            # out_sem==16*(N+1) proves every wait in the program has been
            # passed and every DMA has retired; clear sems so the loaded NEFF
            # can be re-executed (PJRT keeps it loaded across kernel() calls).
            sems = (in_sem, mm_sem, cp_sem, comp_sem, out_sem)
            nums = sorted(s.num for s in sems)
            if nums == list(range(nums[0], nums[0] + len(nums))):
                scalar.sem_clear(range(nums[0], nums[-1] + 1))
            else:
                for s in sems:
                    scalar.sem_clear(s)

    _NC_CACHE[variant] = nc
    return nc


FCOLS = 64  # fence re-read width (cols); sets chunk 7's byte-landing margin


def _build_nc_v5(variant="v5"):
    """v5/v6/v7: bf16 pipeline with every DMA-data gate on that DMA's OWN
    semaphore. A `.then_inc(sem, 16)` DMA completes as 16 independent
    per-ring sub-DMAs; a shared counter only bounds the TOTAL inc count, so
    a straggler ring can leave one chunk's slice unwritten while later
    chunks' rings push the total past the gate (observed on hardware as
    stale partitions 96..103 = ring 12's slice). Gating each multiply on
    chunk n+1's own sem==16 proves every ring finished chunk n+1, and
    per-ring in-order processing then guarantees chunk n's bytes landed a
    full transfer earlier. A 64-col re-read fence with its own sem does the
    same for chunk 7, and the PE gates on wt_sem==32 (both wt loads fully
    complete; the duplicate load makes any in-flight bytes value-identical
    for the lhsT columns the PE reads first).

    v6: W chain latency cuts — 4x256-col matmuls (PE p-state: only the
    first runs cold) and a 2x512-col Act copy so the first multiply gates
    on the first half of wsb only.
    v7: v6 + both wt loads move to the Pool/SWDGE ring (the second loads
    only the lhsT block) so chunk 0 leads the HWDGE stream: the x chunks
    then run back to back from the earliest possible slot and the small wt
    transfers slot into the queue backlog without costing HWDGE cadence.
    """
    f32 = mybir.dt.float32
    bf16 = mybir.dt.bfloat16
    v7fam = variant in (
        "v7b",
        "v7d",
        "v9",
        "v10",
        "v11",
        "v12",
    ) or variant.startswith("v7c")
    # v10: compact [3, 1152] wt table (per-channel G_c blocks + Bh'^T) and 8
    # per-channel k=3 matmuls instead of the fused k=24 block-diagonal table
    # (whose rhs was 7/8 zeros) — the wt DMA drops from 154ns to 19ns on the
    # saturated DMA device
    k3 = variant in ("v10", "v11", "v12")
    # v11: wider first slice (the longer first output transfer bridges the
    # in->out transition), 32-col fence (its consumer trails by ~3us, so the
    # smaller byte-landing margin is still ample), and outputs alternating
    # between the SP and Act queues so consecutive output DMAs issue in
    # parallel instead of serializing on one sequencer
    S0v = 384 if variant in ("v11", "v12") else S0
    fcols = 32 if variant in ("v11", "v12") else FCOLS
    alternate_outs = variant == "v11"
    split_w = variant in ("v6", "v7") or v7fam or variant.startswith("v8")
    pool_wt = variant == "v7" or v7fam or variant.startswith("v8")
    single_wt = v7fam or variant.startswith("v8")
    # first N output DMAs issued from the SP queue instead of Act
    sp_outs = {
        "v7c": 2,
        "v7c3": 3,
        "v7c4": 4,
        "v7c9": 9,
        "v9": 4,
        "v10": 4,
        "v11": 4,
        "v12": 4,
    }.get(variant, 0)
    # v8: three-way first-chunk pipelining — the W copy lands in [0:S1] and
    # [S1:FW] halves, multiplies and outputs follow in [0:S0], [S0:S1],
    # [S1:FW] slices, so the first three output DMAs queue before the input
    # stream drains
    three_way = variant.startswith("v8")
    S1 = 512 if variant == "v8b" else 768
    n_mm = 8 if variant == "v7d" else (4 if split_w else 2)
    n_cp = 2 if split_w else 1
    if k3:
        n_mm = CS
    wt_rows = KH if k3 else KF
    nc = bass.Bass()
    xt = nc.dram_tensor("xt", [H, FX], bf16, kind="ExternalInput")
    wt = nc.dram_tensor("wt", [wt_rows, WTC], bf16, kind="ExternalInput")
    out = nc.dram_tensor("out", [H, FX], bf16, kind="ExternalOutput")

    from contextlib import ExitStack

    with ExitStack() as stack:
        block = stack.enter_context(nc.Block())
        wt_sem = stack.enter_context(nc.semaphore("wt_sem"))
        ch_sem = [
            stack.enter_context(nc.semaphore(f"ch_sem{n}")) for n in range(N)
        ]
        f_sem = stack.enter_context(nc.semaphore("f_sem"))
        mm_sem = stack.enter_context(nc.semaphore("mm_sem"))
        cp_sem = stack.enter_context(nc.semaphore("cp_sem"))
        comp_sem = stack.enter_context(nc.semaphore("comp_sem"))
        out_sem = stack.enter_context(nc.semaphore("out_sem"))
        wib = stack.enter_context(nc.sbuf_tensor("wib", [wt_rows, WTC], bf16))
        spacer = stack.enter_context(nc.sbuf_tensor("spacer", [1, 1], bf16))
        fence_buf = stack.enter_context(
            nc.sbuf_tensor("fence_buf", [H, fcols], bf16)
        )
        xbuf = stack.enter_context(nc.sbuf_tensor("xbuf", [H, FX], bf16))
        ybuf = stack.enter_context(nc.sbuf_tensor("ybuf", [H, FX], bf16))
        wsb = stack.enter_context(nc.sbuf_tensor("wsb", [H, FW], bf16))
        WP = stack.enter_context(nc.psum_tensor("WP", [H, FW], f32))

        # multiply jobs in issue order: (lo, hi, cp_gate, chunk_gate) where
        # cp_gate is a cp_sem threshold (0 = covered by program order) and
        # chunk_gate is a chunk-sem index carried by a spacer copy (None =
        # already covered). comp_sem increments in this order, so the
        # matching output jobs gate on position+1.
        if variant == "v12":
            # interleave chunk-1 half-multiplies around the copy2 wait: DVE
            # never idles between the W-copy halves and the first four output
            # DMAs are all queued before the input stream drains. mul_1a
            # carries the chunk-2 gate itself (cp1 passed via program order);
            # mul_0b carries the copy2 gate; mul_1b needs neither.
            half1 = FW + FW // 2
            mul_jobs = [
                (0, S0v, 1, None),
                (FW, half1, 0, 2),
                (S0v, FW, 2, None),
                (half1, 2 * FW, 0, None),
            ] + [(n * FW, (n + 1) * FW, 0, n + 1) for n in range(2, N)]
        elif variant == "v9":
            # chunk 1's multiply runs between the two chunk-0 slices so its
            # full-size output DMA is ready sooner
            mul_jobs = [
                (0, S0v, 1, None),
                (FW, 2 * FW, n_cp, 2),
                (S0v, FW, 0, None),
            ] + [((n) * FW, (n + 1) * FW, 0, n + 1) for n in range(2, N)]
        else:
            if three_way:
                mul0 = [(0, S0v, 1, None), (S0v, S1, 0, None), (S1, FW, 2, None)]
            else:
                mul0 = [
                    (0, S0v, 1, None),
                    (S0v, FW, n_cp if n_cp > 1 else 0, None),
                ]
            mul_jobs = mul0 + [
                (n * FW, (n + 1) * FW, 0, n + 1) for n in range(1, N)
            ]
        # output DMA jobs (slice, comp_sem gate) in multiply-completion
        # order, skipping the [0:S0] slice (issued from the Pool queue); the
        # first sp_outs are issued from the SP queue (free earlier, shorter
        # DGE delay), the rest from Act
        out_jobs = [
            (lo, hi, i + 1)
            for i, (lo, hi, _, _) in enumerate(mul_jobs)
            if not (lo == 0 and hi == S0v)
        ]
        if alternate_outs:
            sp_jobs, act_jobs = out_jobs[0::2], out_jobs[1::2]
        else:
            sp_jobs, act_jobs = out_jobs[:sp_outs], out_jobs[sp_outs:]

        @block.sync
        def _(sync: bass.BassEngine):
            if not pool_wt:
                sync.dma_start(out=wib[:, :], in_=wt[:, :]).then_inc(wt_sem, 16)
            for n in range(N):
                sl = slice(n * FW, (n + 1) * FW)
                sync.dma_start(out=xbuf[:, sl], in_=xt[:, sl]).then_inc(
                    ch_sem[n], 16
                )
            # fence: re-read chunk 7's tail columns through every ring so its
            # own-sem gate also carries a byte-landing margin
            sync.dma_start(
                out=fence_buf[:, :], in_=xbuf[:, FX - fcols : FX]
            ).then_inc(f_sem, 16)
            for lo, hi, comp_need in sp_jobs:
                sync.wait_ge(comp_sem, comp_need)
                sync.dma_start(out=out[:, lo:hi], in_=ybuf[:, lo:hi]).then_inc(
                    out_sem, 16
                )

        @block.gpsimd
        def _(gpsimd: bass.BassEngine):
            # wt load(s) on the SWDGE ring: Pool desc-gen costs no HWDGE
            # cadence. single_wt: PE gates on wt_sem==16 (all rings done)
            # plus its throwaway matmul; otherwise wt_sem==32 proves both
            # loads complete and duplicate bytes make any race harmless.
            gpsimd.dma_start(out=wib[:, :], in_=wt[:, :]).then_inc(wt_sem, 16)
            if pool_wt and not single_wt:
                gpsimd.dma_start(out=wib[:, 0:H], in_=wt[:, 0:H]).then_inc(
                    wt_sem, 16
                )
            gpsimd.wait_ge(comp_sem, 1)
            gpsimd.dma_start(out=out[:, 0:S0v], in_=ybuf[:, 0:S0v]).then_inc(
                out_sem, 16
            )

        @block.tensor
        def _(tensor: bass.BassEngine):
            step = FW // n_mm
            # k3: lhsT (Bh'^T) sits at cols FW:FW+H, rhs blocks G_c at cols
            # c*W:(c+1)*W; k24: lhsT at cols 0:H, rhs blocks at H:H+FW
            lhsT = wib[:, FW : FW + H] if k3 else wib[:, 0:H]
            rhs_off = 0 if k3 else H
            if single_wt:
                # gate on the single wt load's own sem (all 16 rings done);
                # the throwaway 1-column matmul soaks up the remaining
                # sem-vs-byte-visibility skew before the real matmuls read wib
                tensor.wait_ge(wt_sem, 16)
                tensor.matmul(
                    WP[:, 0:1],
                    lhsT,
                    wib[:, rhs_off : rhs_off + 1],
                    start=True,
                    stop=True,
                )
            else:
                tensor.wait_ge(wt_sem, 32)
            for m in range(n_mm):
                # W[:, m*step:(m+1)*step] = lhsT^T @ rhs_slice -> PSUM f32;
                # <=512-col matmuls keep each output inside one PSUM bank
                tensor.matmul(
                    WP[:, m * step : (m + 1) * step],
                    lhsT,
                    wib[:, rhs_off + m * step : rhs_off + (m + 1) * step],
                    start=True,
                    stop=True,
                ).then_inc(mm_sem, 1)

        @block.vector
        def _(vector: bass.BassEngine):
            # spacer carries the chunk-1 wait (chunk 0's margin) so the first
            # multiply only needs the W-ready wait
            vector.wait_ge(ch_sem[1], 16)
            vector.tensor_copy(spacer[:, :], xbuf[0:1, FW - 1 : FW])
            for lo, hi, cp_need, chunk_gate in mul_jobs:
                if chunk_gate is not None:
                    gate = ch_sem[chunk_gate] if chunk_gate < N else f_sem
                    if cp_need:
                        # both gates needed: a spacer copy carries the chunk
                        # wait so the multiply itself has only the cp wait
                        vector.wait_ge(gate, 16)
                        vector.tensor_copy(
                            spacer[:, :], xbuf[0:1, FW - 1 : FW]
                        )
                    else:
                        vector.wait_ge(gate, 16)
                if cp_need:
                    vector.wait_ge(cp_sem, cp_need)
                wlo = lo % FW
                vector.tensor_mul(
                    ybuf[:, lo:hi], xbuf[:, lo:hi], wsb[:, wlo : wlo + hi - lo]
                ).then_inc(comp_sem, 1)

        @block.scalar
        def _(scalar: bass.BassEngine):
            # narrow W to bf16 in SBUF (Act engine, f32 PSUM read) so the
            # multiplies run in DVE 2x mode and DVE never stalls on this copy
            if three_way:
                copies = [(0, S1, (S1 * n_mm) // FW + (1 if (S1 * n_mm) % FW else 0)), (S1, FW, n_mm)]
            else:
                cstep = FW // n_cp
                copies = [
                    (j * cstep, (j + 1) * cstep, (j + 1) * (n_mm // n_cp))
                    for j in range(n_cp)
                ]
            for lo, hi, mm_need in copies:
                scalar.wait_ge(mm_sem, mm_need)
                scalar.activation(
                    wsb[:, lo:hi],
                    WP[:, lo:hi],
                    mybir.ActivationFunctionType.Copy,
                ).then_inc(cp_sem, 1)
            for lo, hi, comp_need in act_jobs:
                scalar.wait_ge(comp_sem, comp_need)
                scalar.dma_start(out=out[:, lo:hi], in_=ybuf[:, lo:hi]).then_inc(
                    out_sem, 16
                )
            scalar.wait_ge(out_sem, 16 * (len(out_jobs) + 1))
            # out_sem==16*(N+1) proves every wait in the program has been
            # passed and every DMA has retired; clear sems so the loaded NEFF
            # can be re-executed (PJRT keeps it loaded across kernel() calls).
            sems = (
                wt_sem,
                *ch_sem,
                f_sem,
                mm_sem,
                cp_sem,
                comp_sem,
                out_sem,
            )
            nums = sorted(s.num for s in sems)
            if nums == list(range(nums[0], nums[0] + len(nums))):
                scalar.sem_clear(range(nums[0], nums[-1] + 1))
            else:
                for s in sems:
                    scalar.sem_clear(s)

    return nc


def _host_tables(R: np.ndarray, k3: bool = True):
    """Per-core bf16 matmul tables (built in f64, cast to bf16).

    k3 layout [3, 1152]: cols c*W:(c+1)*W hold G_c = S_c @ Bw'^T and cols
    FW:FW+H hold lhsT = Bh'^T; each channel's W block is lhsT^T @ G_c.

    k24 layout [24, 1152]: cols 0:H hold lhsT[(c,i), h] = Bh'[h, i] (tiled
    over c) and cols H: hold the block-diagonal rhs[(c,i), c*W+w] =
    G_c[i, w], so one fused matmul pair gives W_core = lhsT^T @ rhs.
    """
    s = np.asarray(R, np.float64).sum(axis=1).reshape(C, KH, KW)
    idx = np.arange(H)
    lo = np.maximum(0, idx - (H - KH))
    hi = np.minimum(KH - 1, idx)
    B = (
        (np.arange(KH)[None, :] >= lo[:, None])
        & (np.arange(KH)[None, :] <= hi[:, None])
    ).astype(np.float64)
    Bp = B / (hi - lo + 1)[:, None]  # [H, 3] = Bh' == Bw' (H == W, KH == KW)
    G = np.einsum("cij,wj->ciw", s, Bp)  # [C, 3, W]: S_c @ Bw'^T
    tables = []
    for k in range(NCORES):
        if k3:
            t = np.zeros((KH, WTC), np.float64)
            for c in range(CS):
                t[:, c * W : (c + 1) * W] = G[k * CS + c]
            t[:, FW : FW + H] = Bp.T
        else:
            t = np.zeros((KF, WTC), np.float64)
            t[:, :H] = np.tile(Bp.T, (CS, 1))
            for c in range(CS):
                t[c * KH : (c + 1) * KH, H + c * W : H + (c + 1) * W] = G[
                    k * CS + c
                ]
        tables.append(t.astype(BF16))
    return tables


def _make_in_maps(x: np.ndarray, R: np.ndarray, k3: bool = True):
    tables = _host_tables(R, k3=k3)
    xT = np.ascontiguousarray(
        np.asarray(x, dtype=np.float32).transpose(2, 0, 1, 3)
    )  # [H, N, C, W]
    in_maps = []
    for k in range(NCORES):
        xs = np.ascontiguousarray(
            xT[:, :, k * CS : (k + 1) * CS, :]
        ).reshape(H, FX).astype(BF16)
        in_maps.append({"xt": xs, "wt": tables[k]})
    return in_maps


DEFAULT_VARIANT = "v10"


def kernel(x, R):
    in_maps = _make_in_maps(x, R, k3=DEFAULT_VARIANT == "v10")
    nc = _build_nc(DEFAULT_VARIANT)
    res = run_bass_kernel_spmd(nc, in_maps, core_ids=list(range(NCORES)))

    out = np.empty((N, C, H, W), np.float32)
    for k in range(NCORES):
        blk = (
            np.asarray(res.results[k]["out"])
            .astype(np.float32)
            .reshape(H, N, CS, W)
            .transpose(1, 2, 0, 3)
        )
        out[:, k * CS : (k + 1) * CS] = blk
    return out
